# revision 6
# baseline (speedup 1.0000x reference)
"""AttentionPool Trainium2 kernel.

Computes, for x (B,T,m), W1 (m,m), W2 (m,m), vm (1,m):
    h      = tanh(x @ W1 + vm @ W2)          (B,T,m)
    scores = h @ vm[0]                       (B,T,1)
    w      = softmax(scores, axis=T)
    out    = sum(x * w, axis=T, keepdims)    (B,1,m)

Sharding: data-parallel over B across 8 NeuronCores (2 examples per core);
W1/W2/vm replicated.  Softmax uses no max-subtraction: |scores| <= ||vm||_1
(~13 for this problem scale), safely inside fp32 exp range, so the kernel
is a single streaming pass over x with exp/Z accumulated online.

Per-core dataflow (chunk = 512 rows of T):
  DMA x chunk (natural f32) -> cast bf16 (DVE) -> PE transpose -> xT (SBUF)
  -> h^T = W1.T @ x.T per n-half (PE, bf16) -> tanh+bias (ACT, per-partition
  bias in h^T layout) -> scores via h-as-stationary matmuls (PE; lands
  t-partitioned) -> exp + per-chunk Z (ACT) -> pooling matmuls
  A += e.T @ x_chunk (PE, f32) -> final A/Z normalize + DMA out.
"""

import numpy as np

import concourse.bass as bass
import concourse.tile as tile
from concourse import bacc, mybir
from concourse.bass_utils import run_bass_kernel_spmd
from concourse.masks import make_identity

FP32 = mybir.dt.float32
BF16 = mybir.dt.float16  # score-path dtype: fp16 (1 cyc/row on PE, 11-bit mantissa)
AF = mybir.ActivationFunctionType

N_CORES = 8
B = 16
B_PER_CORE = B // N_CORES  # 2
T = 8192
M = 256
P = 128
CHUNK = 512          # t rows per chunk
NT = CHUNK // P      # 4 t-tiles per chunk
NCHUNK = T // CHUNK  # 16 chunks per example


def _build_program() -> bass.Bass:
    nc = bacc.Bacc("TRN2", target_bir_lowering=False, debug=False)

    x = nc.dram_tensor("x", [B_PER_CORE, T, M], FP32, kind="ExternalInput")
    W1 = nc.dram_tensor("W1", [M, M], FP32, kind="ExternalInput")
    W2 = nc.dram_tensor("W2", [M, M], FP32, kind="ExternalInput")
    vm = nc.dram_tensor("vm", [1, M], FP32, kind="ExternalInput")
    out = nc.dram_tensor("out", [B_PER_CORE, M], FP32, kind="ExternalOutput")

    with tile.TileContext(nc) as tc:
        with (
            tc.tile_pool(name="setup", bufs=1) as setup,
            tc.tile_pool(name="xin", bufs=6) as xin_pool,
            tc.tile_pool(name="xbf", bufs=2) as xbf_pool,
            tc.tile_pool(name="xtp", bufs=2, space="PSUM") as xtp_pool,
            tc.tile_pool(name="xts", bufs=2) as xts_pool,
            tc.tile_pool(name="hps", bufs=2, space="PSUM") as hps_pool,
            tc.tile_pool(name="hsb", bufs=2) as hsb_pool,
            tc.tile_pool(name="sps", bufs=1, space="PSUM") as sps_pool,
            tc.tile_pool(name="aps", bufs=1, space="PSUM") as aps_pool,
            tc.tile_pool(name="eee", bufs=2) as e_pool,
            tc.tile_pool(name="zzz", bufs=2) as z_pool,
            tc.tile_pool(name="fin", bufs=2) as fin_pool,
        ):
            # ---------------- setup ----------------
            ident = setup.tile([P, P], BF16)
            make_identity(nc, ident)

            # W1 blocks: w1b[p, mh, n] = W1[mh*128+p, n], cast to bf16
            w1f = setup.tile([P, 2, M], FP32)
            nc.sync.dma_start(out=w1f, in_=W1.rearrange("(a p) n -> p a n", p=P))
            w1b = setup.tile([P, 2, M], BF16)
            nc.vector.tensor_copy(w1b, w1f)

            # W2 blocks (f32, setup only)
            w2f = setup.tile([P, 2, M], FP32)
            nc.sync.dma_start(out=w2f, in_=W2.rearrange("(a p) n -> p a n", p=P))

            # vm transposed: vmt[p, mh] = vm[0, mh*128+p]
            vmt_f = setup.tile([P, 2], FP32)
            nc.sync.dma_start(out=vmt_f, in_=vm[0].rearrange("(a p) -> p a", p=P))
            vmt_b = setup.tile([P, 2], BF16)
            nc.vector.tensor_copy(vmt_b, vmt_f)

            # c = vm @ W2, computed directly transposed: c_sb[p, nh] = c[nh*128+p]
            c_ps = sps_pool.tile([P, 2], FP32, tag="sps")
            for nh in range(2):
                for mh in range(2):
                    nc.tensor.matmul(
                        c_ps[:, nh : nh + 1],
                        lhsT=w2f[:, mh, nh * P : (nh + 1) * P],
                        rhs=vmt_f[:, mh : mh + 1],
                        start=(mh == 0),
                        stop=(mh == 1),
                    )
            c_sb = setup.tile([P, 2], FP32)
            nc.vector.tensor_copy(c_sb, c_ps)

            ones_col = setup.tile([P, 1], FP32)
            nc.vector.memset(ones_col, 1.0)

            # ---------------- main loop ----------------
            for b in range(B_PER_CORE):
                a_ps = aps_pool.tile([1, M], FP32)
                z_acc = z_pool.tile([P, NCHUNK], FP32)

                for c in range(NCHUNK):
                    # x chunk, natural layout: [p, i, m], t = c*512 + i*128 + p
                    xin = xin_pool.tile([P, NT, M], FP32)
                    nc.sync.dma_start(
                        out=xin,
                        in_=x[b, c * CHUNK : (c + 1) * CHUNK, :].rearrange(
                            "(i p) m -> p i m", p=P
                        ),
                    )

                    # cast to bf16 for the score path
                    xbf = xbf_pool.tile([P, NT, M], BF16)
                    nc.vector.tensor_copy(xbf, xin)

                    # PE transpose -> xT psum tile: xtp[p, mh, i, q] = x[t=i*128+q, mh*128+p]
                    xtp = xtp_pool.tile([P, 2, NT, P], BF16)
                    for i in range(NT):
                        for mh in range(2):
                            nc.tensor.transpose(
                                xtp[:, mh, i, :],
                                xbf[:, i, mh * P : (mh + 1) * P],
                                ident,
                            )
                    xts = xts_pool.tile([P, 2, NT, P], BF16)
                    nc.vector.tensor_copy(xts, xtp)

                    # h^T = W1.T @ x^T  (per n-half), accumulate over m-halves
                    hps = hps_pool.tile([P, 2, CHUNK], FP32)
                    for nh in range(2):
                        for mh in range(2):
                            nc.tensor.matmul(
                                hps[:, nh, :],
                                lhsT=w1b[:, mh, nh * P : (nh + 1) * P],
                                rhs=xts[:, mh],
                                start=(mh == 0),
                                stop=(mh == 1),
                            )

                    # tanh with per-partition bias c
                    hsb = hsb_pool.tile([P, 2, CHUNK], BF16)
                    for nh in range(2):
                        nc.scalar.activation(
                            hsb[:, nh],
                            hps[:, nh],
                            AF.Tanh,
                            bias=c_sb[:, nh : nh + 1],
                        )

                    # scores: s[t] = sum_n vm[n] h[n, t], t-partitioned output
                    sps = sps_pool.tile([P, NT], FP32, tag="sps")
                    for i in range(NT):
                        for nh in range(2):
                            nc.tensor.matmul(
                                sps[:, i : i + 1],
                                lhsT=hsb[:, nh, i * P : (i + 1) * P],
                                rhs=vmt_b[:, nh : nh + 1],
                                start=(nh == 0),
                                stop=(nh == 1),
                            )

                    # e = exp(s), z partial sums per chunk
                    e_sb = e_pool.tile([P, NT], FP32)
                    nc.scalar.activation(
                        e_sb,
                        sps,
                        AF.Exp,
                        accum_out=z_acc[:, c : c + 1],
                    )

                    # pooling: A += e_tile.T @ x_tile
                    for i in range(NT):
                        nc.tensor.matmul(
                            a_ps,
                            lhsT=e_sb[:, i : i + 1],
                            rhs=xin[:, i],
                            start=(c == 0 and i == 0),
                            stop=(c == NCHUNK - 1 and i == NT - 1),
                        )

                # ---- finalize example ----
                z_red = fin_pool.tile([P, 1], FP32)
                nc.vector.reduce_sum(z_red, z_acc, axis=mybir.AxisListType.X)
                z_ps = sps_pool.tile([1, 1], FP32, tag="sps")
                nc.tensor.matmul(z_ps, lhsT=z_red, rhs=ones_col, start=True, stop=True)
                rz = fin_pool.tile([1, 1], FP32)
                nc.vector.reciprocal(rz, z_ps)
                outsb = fin_pool.tile([1, M], FP32)
                nc.vector.tensor_scalar_mul(outsb, a_ps, rz)
                nc.sync.dma_start(out=out[b : b + 1, :], in_=outsb)

    return nc


_PROGRAM_CACHE: list = []


def _get_program() -> bass.Bass:
    if not _PROGRAM_CACHE:
        nc = _build_program()
        nc.finalize()
        _PROGRAM_CACHE.append(nc)
    return _PROGRAM_CACHE[0]


def kernel(x, W1, W2, vm):
    x = np.ascontiguousarray(x, dtype=np.float32)
    W1 = np.ascontiguousarray(W1, dtype=np.float32)
    W2 = np.ascontiguousarray(W2, dtype=np.float32)
    vm = np.ascontiguousarray(vm, dtype=np.float32)

    nc = _get_program()
    core_ids = list(range(N_CORES))
    in_maps = [
        {
            "x": x[i * B_PER_CORE : (i + 1) * B_PER_CORE],
            "W1": W1,
            "W2": W2,
            "vm": vm,
        }
        for i in range(N_CORES)
    ]
    res = run_bass_kernel_spmd(nc, in_maps, core_ids)
    out = np.concatenate([res.results[i]["out"] for i in range(N_CORES)], axis=0)
    return out.reshape(B, 1, M)
